# revision 14
# baseline (speedup 1.0000x reference)
"""Distributed Trainium2 kernel for the CHMM ratio-matmul problem.

Computes out = ratio @ cp_e where
    ll    = max(cp, axis=-1)                      # [B]
    ratio = pf * exp(ll - pp)                     # [I,B]  (== pf / exp(pp - ll))
    cp_e  = exp(cp - ll[:, None])                 # [B,J]

Shapes: pf, pp [1048576, 32] f32; cp [32, 32] f32; out [1048576, 32] f32.

Sharding: the I axis is split across 8 NeuronCores (pure data parallel,
no communication).  Each core's shard is laid out host-side with B on
the SBUF partition axis: partition 32*q + b holds pf[q*CHUNK + i, b]
for i in [0, CHUNK).  This makes the contraction axis (B) the partition
axis, so the TensorEngine streams the data with cp_e as the stationary
operand and no on-device transposes are needed.  The matmul output is
[J, I]-major per group; the host reassembles the natural [I, J] layout.
"""

import os
import sys

import numpy as np

if "/opt/trn_rl_repo" not in sys.path:
    sys.path.insert(0, "/opt/trn_rl_repo")

I, B, J = 1048576, 32, 32
NCORES = 8
RPC = I // NCORES          # 131072 rows per core
NGRP = 4                   # partition groups of 32 (B) each
CHUNK = RPC // NGRP        # 32768 free-dim elements per partition
TILE_F = 2048              # free-dim tile size (1 MiB per [128, TILE_F] f32 tile)
N_TILES = CHUNK // TILE_F  # 16
PSUM_F = 2048              # psum tile free dim (4 banks)
MM_N = 512                 # matmul moving free dim (one PSUM bank of f32)

LAST_EXEC_TIME_NS = None
LAST_RESULTS = None

_AXON_SO = "/opt/axon/libaxon_pjrt.so"


def _ensure_ntff_hook():
    """Provide antenv.axon_hooks (NTFF profiling hook) if the image's
    antenv package lacks it, via direct ctypes calls into the axon .so.
    Mirrors trn_agent_boot._ntff_profile_via_ctypes."""
    try:
        from antenv.axon_hooks import get_axon_ntff_profile_hook  # noqa: F401

        return
    except ImportError:
        pass

    import contextlib
    import ctypes
    import types

    lib = ctypes.CDLL(_AXON_SO)
    if not hasattr(lib, "axon_start_nrt_profile"):
        return
    lib.axon_start_nrt_profile.argtypes = [
        ctypes.POINTER(ctypes.c_int64),
        ctypes.c_size_t,
    ]
    lib.axon_start_nrt_profile.restype = ctypes.c_int64
    lib.axon_stop_nrt_profile.argtypes = [ctypes.c_char_p]
    lib.axon_stop_nrt_profile.restype = ctypes.c_int64

    @contextlib.contextmanager
    def _hook(output_dir, device_ids):
        import jax

        jax.devices()
        if device_ids:
            ids = (ctypes.c_int64 * len(device_ids))(*device_ids)
            rc = lib.axon_start_nrt_profile(ids, len(device_ids))
        else:
            rc = lib.axon_start_nrt_profile(None, 0)
        if rc != 0:
            raise RuntimeError(f"axon_start_nrt_profile rc={rc}")
        try:
            yield
        finally:
            n = lib.axon_stop_nrt_profile(str(output_dir).encode())
            print(f"ntff profile: {n} file(s) written to {output_dir}", file=sys.stderr)

    mod = types.ModuleType("antenv.axon_hooks")
    mod.get_axon_ntff_profile_hook = lambda: _hook
    mod.set_axon_ntff_profile_hook = lambda h: None
    sys.modules["antenv.axon_hooks"] = mod
    import antenv

    antenv.axon_hooks = mod


def _build_nc():
    from concourse import bacc, bass, tile
    from concourse import mybir

    f32 = mybir.dt.float32
    nc = bacc.Bacc()

    pf_ext = nc.declare_dram_parameter("pft", [128, CHUNK], f32, isOutput=False)
    pp_ext = nc.declare_dram_parameter("ppt", [128, CHUNK], f32, isOutput=False)
    cp_ext = nc.declare_dram_parameter("cp", [128, J], f32, isOutput=False)
    out_ext = nc.declare_dram_parameter("out", [128, CHUNK], f32, isOutput=True)

    with tile.TileContext(nc) as tc:
        with (
            tc.tile_pool(name="const", bufs=1) as const_pool,
            tc.tile_pool(name="pf", bufs=4) as pf_pool,
            tc.tile_pool(name="pp", bufs=4) as pp_pool,
            tc.tile_pool(name="work", bufs=3) as work_pool,
            tc.tile_pool(name="outs", bufs=3) as out_pool,
            tc.tile_pool(name="psum", bufs=2, space="PSUM") as psum_pool,
        ):
            # First tiles' input DMAs lead the program so the DMA queues
            # fill before the (dependency-light) cp prologue runs.
            pp_tiles = {}
            pf_tiles = {}
            for t in range(2):
                pp_tiles[t] = pp_pool.tile([128, TILE_F], f32, tag="pp", name="pp_t")
                nc.sync.dma_start(pp_tiles[t][:], pp_ext[:, bass.ts(t, TILE_F)])
                pf_tiles[t] = pf_pool.tile([128, TILE_F], f32, tag="pf", name="pf_t")
                nc.sync.dma_start(pf_tiles[t][:], pf_ext[:, bass.ts(t, TILE_F)])

            # Prologue: cp (replicated x4 on partitions) -> ll, cp_e
            cp_t = const_pool.tile([128, J], f32)
            nc.sync.dma_start(cp_t[:], cp_ext[:])
            ll = const_pool.tile([128, 1], f32)
            nc.vector.tensor_reduce(
                ll[:], cp_t[:], axis=mybir.AxisListType.X, op=mybir.AluOpType.max
            )
            nll = const_pool.tile([128, 1], f32)
            nc.vector.tensor_scalar_mul(nll[:], ll[:], -1.0)
            cpe = const_pool.tile([128, J], f32)
            nc.scalar.activation(
                cpe[:], cp_t[:], mybir.ActivationFunctionType.Exp, bias=nll[:]
            )

            def compute_span(pp_src, pf_src, ps, o_t, f0, width, t, out_f0):
                """exp+mult+matmul+copy for columns [f0, f0+width) of the
                given input APs, writing psum/staging at the same offset,
                then DMA staging out (caller controls granularity via o_t)."""
                e_t = work_pool.tile([128, width], f32, tag=f"e{width}", name="e_t")
                nc.scalar.activation(
                    e_t[:],
                    pp_src[:, f0 : f0 + width],
                    mybir.ActivationFunctionType.Exp,
                    bias=ll[:],
                    scale=-1.0,
                )
                r_t = work_pool.tile([128, width], f32, tag=f"r{width}", name="r_t")
                nc.vector.tensor_mul(r_t[:], pf_src[:, f0 : f0 + width], e_t[:])

                for n in range(width // MM_N):
                    for q in range(NGRP):
                        p0 = 32 * q
                        nc.tensor.matmul(
                            ps[p0 : p0 + 32, f0 + n * MM_N : f0 + (n + 1) * MM_N],
                            cpe[p0 : p0 + 32, :],
                            r_t[p0 : p0 + 32, bass.ts(n, MM_N)],
                            start=True,
                            stop=True,
                            tile_position=(p0, p0),
                        )
                for n in range(width // MM_N):
                    src = ps[:, f0 + n * MM_N : f0 + (n + 1) * MM_N]
                    dst = o_t[:, f0 + n * MM_N : f0 + (n + 1) * MM_N]
                    if n % 2 == 0:
                        nc.vector.tensor_copy(dst, src)
                    else:
                        nc.scalar.copy(dst, src)
                # Output DMAs ride the ACT HWDGE ring so they don't queue
                # behind input DMAs on the sync ring.
                nc.scalar.dma_start(
                    out_ext[:, out_f0 + f0 : out_f0 + f0 + width],
                    o_t[:, f0 : f0 + width],
                )

            for t in range(N_TILES - 1):
                if t not in pp_tiles:
                    pp_tiles[t] = pp_pool.tile([128, TILE_F], f32, tag="pp", name="pp_t")
                    nc.sync.dma_start(pp_tiles[t][:], pp_ext[:, bass.ts(t, TILE_F)])
                    pf_tiles[t] = pf_pool.tile([128, TILE_F], f32, tag="pf", name="pf_t")
                    nc.sync.dma_start(pf_tiles[t][:], pf_ext[:, bass.ts(t, TILE_F)])
                pp_t, pf_t = pp_tiles.pop(t), pf_tiles.pop(t)
                ps = psum_pool.tile([128, PSUM_F], f32, tag="ps", name="ps")
                o_t = out_pool.tile([128, TILE_F], f32, tag="o", name="o_t")
                compute_span(pp_t, pf_t, ps, o_t, 0, PSUM_F, t, t * TILE_F)

            # Last tile at fine granularity: split input DMAs + 512-wide
            # compute chunks shorten the pipeline drain after the final
            # input bytes land.
            t = N_TILES - 1
            ps = psum_pool.tile([128, PSUM_F], f32, tag="ps", name="ps")
            o_t = out_pool.tile([128, TILE_F], f32, tag="o", name="o_t")
            for c in range(TILE_F // MM_N):
                pp_c = pp_pool.tile([128, MM_N], f32, tag="pp_s", name="pp_c")
                nc.sync.dma_start(
                    pp_c[:], pp_ext[:, t * TILE_F + c * MM_N : t * TILE_F + (c + 1) * MM_N]
                )
                pf_c = pf_pool.tile([128, MM_N], f32, tag="pf_s", name="pf_c")
                nc.sync.dma_start(
                    pf_c[:], pf_ext[:, t * TILE_F + c * MM_N : t * TILE_F + (c + 1) * MM_N]
                )
                # compute_span with f0 referencing ps/o_t coords; inputs are
                # standalone chunk tiles, so wrap with an offset shim.
                e_t = work_pool.tile([128, MM_N], f32, tag="e_s", name="e_t")
                nc.scalar.activation(
                    e_t[:], pp_c[:], mybir.ActivationFunctionType.Exp,
                    bias=ll[:], scale=-1.0,
                )
                r_t = work_pool.tile([128, MM_N], f32, tag="r_s", name="r_t")
                nc.vector.tensor_mul(r_t[:], pf_c[:], e_t[:])
                for q in range(NGRP):
                    p0 = 32 * q
                    nc.tensor.matmul(
                        ps[p0 : p0 + 32, bass.ts(c, MM_N)],
                        cpe[p0 : p0 + 32, :],
                        r_t[p0 : p0 + 32, :],
                        start=True,
                        stop=True,
                        tile_position=(p0, p0),
                    )
                src = ps[:, bass.ts(c, MM_N)]
                dst = o_t[:, bass.ts(c, MM_N)]
                if c % 2 == 0:
                    nc.vector.tensor_copy(dst, src)
                else:
                    nc.scalar.copy(dst, src)
                nc.scalar.dma_start(
                    out_ext[:, t * TILE_F + c * MM_N : t * TILE_F + (c + 1) * MM_N],
                    o_t[:, bass.ts(c, MM_N)],
                )

    return nc


def _shard_transposed(x: np.ndarray, k: int) -> np.ndarray:
    """Shard rows [k*RPC, (k+1)*RPC) and lay out as [128, CHUNK] with
    partition 32*q + b = x[k*RPC + q*CHUNK + i, b]."""
    shard = x[k * RPC : (k + 1) * RPC, :]
    return np.ascontiguousarray(
        shard.reshape(NGRP, CHUNK, B).transpose(0, 2, 1).reshape(128, CHUNK)
    )


def kernel(pf: np.ndarray, pp: np.ndarray, cp: np.ndarray) -> np.ndarray:
    global LAST_EXEC_TIME_NS, LAST_RESULTS
    from concourse.bass_utils import run_bass_kernel_spmd

    pf = np.ascontiguousarray(np.asarray(pf, dtype=np.float32))
    pp = np.ascontiguousarray(np.asarray(pp, dtype=np.float32))
    cp = np.ascontiguousarray(np.asarray(cp, dtype=np.float32))

    cp_rep = np.ascontiguousarray(np.tile(cp, (NGRP, 1)))
    in_maps = [
        {
            "pft": _shard_transposed(pf, k),
            "ppt": _shard_transposed(pp, k),
            "cp": cp_rep,
        }
        for k in range(NCORES)
    ]

    nc = _build_nc()
    nc.finalize()
    trace = os.environ.get("KERNEL_TRACE", "0") == "1"
    if trace:
        _ensure_ntff_hook()
        # Skip the (slow, possibly unavailable) artifact upload.
        import concourse.bass_utils as _bu

        _bu.upload_artifacts = lambda tmpdir: "local://skipped"
    res = run_bass_kernel_spmd(
        nc, in_maps, core_ids=list(range(NCORES)), trace=trace
    )
    LAST_EXEC_TIME_NS = res.exec_time_ns
    LAST_RESULTS = res

    out = np.empty((I, J), dtype=np.float32)
    for k in range(NCORES):
        o = res.results[k]["out"]  # [128, CHUNK]
        out[k * RPC : (k + 1) * RPC, :] = (
            o.reshape(NGRP, B, CHUNK).transpose(0, 2, 1).reshape(RPC, J)
        )
    return out


# revision 15
# speedup vs baseline: 1.0259x; 1.0259x over previous
"""Distributed Trainium2 kernel for the CHMM ratio-matmul problem.

Computes out = ratio @ cp_e where
    ll    = max(cp, axis=-1)                      # [B]
    ratio = pf * exp(ll - pp)                     # [I,B]  (== pf / exp(pp - ll))
    cp_e  = exp(cp - ll[:, None])                 # [B,J]

Shapes: pf, pp [1048576, 32] f32; cp [32, 32] f32; out [1048576, 32] f32.

Sharding: the I axis is split across 8 NeuronCores (pure data parallel,
no communication).  Each core's shard is laid out host-side with B on
the SBUF partition axis: partition 32*q + b holds pf[q*CHUNK + i, b]
for i in [0, CHUNK).  This makes the contraction axis (B) the partition
axis, so the TensorEngine streams the data with cp_e as the stationary
operand and no on-device transposes are needed.  The matmul output is
[J, I]-major per group; the host reassembles the natural [I, J] layout.
"""

import os
import sys

import numpy as np

if "/opt/trn_rl_repo" not in sys.path:
    sys.path.insert(0, "/opt/trn_rl_repo")

I, B, J = 1048576, 32, 32
NCORES = 8
RPC = I // NCORES          # 131072 rows per core
NGRP = 4                   # partition groups of 32 (B) each
CHUNK = RPC // NGRP        # 32768 free-dim elements per partition
TILE_F = 2048              # free-dim tile size (1 MiB per [128, TILE_F] f32 tile)
N_TILES = CHUNK // TILE_F  # 16
PSUM_F = 2048              # psum tile free dim (4 banks)
MM_N = 512                 # matmul moving free dim (one PSUM bank of f32)

LAST_EXEC_TIME_NS = None
LAST_RESULTS = None

_AXON_SO = "/opt/axon/libaxon_pjrt.so"


def _ensure_ntff_hook():
    """Provide antenv.axon_hooks (NTFF profiling hook) if the image's
    antenv package lacks it, via direct ctypes calls into the axon .so.
    Mirrors trn_agent_boot._ntff_profile_via_ctypes."""
    try:
        from antenv.axon_hooks import get_axon_ntff_profile_hook  # noqa: F401

        return
    except ImportError:
        pass

    import contextlib
    import ctypes
    import types

    lib = ctypes.CDLL(_AXON_SO)
    if not hasattr(lib, "axon_start_nrt_profile"):
        return
    lib.axon_start_nrt_profile.argtypes = [
        ctypes.POINTER(ctypes.c_int64),
        ctypes.c_size_t,
    ]
    lib.axon_start_nrt_profile.restype = ctypes.c_int64
    lib.axon_stop_nrt_profile.argtypes = [ctypes.c_char_p]
    lib.axon_stop_nrt_profile.restype = ctypes.c_int64

    @contextlib.contextmanager
    def _hook(output_dir, device_ids):
        import jax

        jax.devices()
        if device_ids:
            ids = (ctypes.c_int64 * len(device_ids))(*device_ids)
            rc = lib.axon_start_nrt_profile(ids, len(device_ids))
        else:
            rc = lib.axon_start_nrt_profile(None, 0)
        if rc != 0:
            raise RuntimeError(f"axon_start_nrt_profile rc={rc}")
        try:
            yield
        finally:
            n = lib.axon_stop_nrt_profile(str(output_dir).encode())
            print(f"ntff profile: {n} file(s) written to {output_dir}", file=sys.stderr)

    mod = types.ModuleType("antenv.axon_hooks")
    mod.get_axon_ntff_profile_hook = lambda: _hook
    mod.set_axon_ntff_profile_hook = lambda h: None
    sys.modules["antenv.axon_hooks"] = mod
    import antenv

    antenv.axon_hooks = mod


def _build_nc():
    from concourse import bacc, bass, tile
    from concourse import mybir

    f32 = mybir.dt.float32
    nc = bacc.Bacc()

    pf_ext = nc.declare_dram_parameter("pft", [128, CHUNK], f32, isOutput=False)
    pp_ext = nc.declare_dram_parameter("ppt", [128, CHUNK], f32, isOutput=False)
    cp_ext = nc.declare_dram_parameter("cp", [128, J], f32, isOutput=False)
    out_ext = nc.declare_dram_parameter("out", [128, CHUNK], f32, isOutput=True)

    with tile.TileContext(nc) as tc:
        with (
            tc.tile_pool(name="const", bufs=1) as const_pool,
            tc.tile_pool(name="pf", bufs=4) as pf_pool,
            tc.tile_pool(name="pp", bufs=4) as pp_pool,
            tc.tile_pool(name="work", bufs=3) as work_pool,
            tc.tile_pool(name="outs", bufs=3) as out_pool,
            tc.tile_pool(name="psum", bufs=2, space="PSUM") as psum_pool,
        ):
            # First tiles' input DMAs lead the program so the DMA queues
            # fill before the (dependency-light) cp prologue runs.
            pp_tiles = {}
            pf_tiles = {}
            for t in range(2):
                pp_tiles[t] = pp_pool.tile([128, TILE_F], f32, tag="pp", name="pp_t")
                nc.sync.dma_start(pp_tiles[t][:], pp_ext[:, bass.ts(t, TILE_F)])
                pf_tiles[t] = pf_pool.tile([128, TILE_F], f32, tag="pf", name="pf_t")
                nc.sync.dma_start(pf_tiles[t][:], pf_ext[:, bass.ts(t, TILE_F)])

            # Prologue: cp (replicated x4 on partitions) -> ll, cp_e
            cp_t = const_pool.tile([128, J], f32)
            nc.sync.dma_start(cp_t[:], cp_ext[:])
            ll = const_pool.tile([128, 1], f32)
            nc.vector.tensor_reduce(
                ll[:], cp_t[:], axis=mybir.AxisListType.X, op=mybir.AluOpType.max
            )
            nll = const_pool.tile([128, 1], f32)
            nc.vector.tensor_scalar_mul(nll[:], ll[:], -1.0)
            cpe = const_pool.tile([128, J], f32)
            nc.scalar.activation(
                cpe[:], cp_t[:], mybir.ActivationFunctionType.Exp, bias=nll[:]
            )

            def compute_span(pp_src, pf_src, ps, o_t, f0, width, t, out_f0):
                """exp+mult+matmul+copy for columns [f0, f0+width) of the
                given input APs, writing psum/staging at the same offset,
                then DMA staging out (caller controls granularity via o_t)."""
                e_t = work_pool.tile([128, width], f32, tag=f"e{width}", name="e_t")
                nc.scalar.activation(
                    e_t[:],
                    pp_src[:, f0 : f0 + width],
                    mybir.ActivationFunctionType.Exp,
                    bias=ll[:],
                    scale=-1.0,
                )
                r_t = work_pool.tile([128, width], f32, tag=f"r{width}", name="r_t")
                nc.vector.tensor_mul(r_t[:], pf_src[:, f0 : f0 + width], e_t[:])

                for n in range(width // MM_N):
                    for q in range(NGRP):
                        p0 = 32 * q
                        nc.tensor.matmul(
                            ps[p0 : p0 + 32, f0 + n * MM_N : f0 + (n + 1) * MM_N],
                            cpe[p0 : p0 + 32, :],
                            r_t[p0 : p0 + 32, bass.ts(n, MM_N)],
                            start=True,
                            stop=True,
                            tile_position=(p0, p0),
                        )
                for n in range(width // MM_N):
                    src = ps[:, f0 + n * MM_N : f0 + (n + 1) * MM_N]
                    dst = o_t[:, f0 + n * MM_N : f0 + (n + 1) * MM_N]
                    if n % 2 == 0:
                        nc.vector.tensor_copy(dst, src)
                    else:
                        nc.scalar.copy(dst, src)
                # Output DMAs ride the ACT HWDGE ring so they don't queue
                # behind input DMAs on the sync ring.
                nc.scalar.dma_start(
                    out_ext[:, out_f0 + f0 : out_f0 + f0 + width],
                    o_t[:, f0 : f0 + width],
                )

            for t in range(N_TILES):
                if t not in pp_tiles:
                    pp_tiles[t] = pp_pool.tile([128, TILE_F], f32, tag="pp", name="pp_t")
                    nc.sync.dma_start(pp_tiles[t][:], pp_ext[:, bass.ts(t, TILE_F)])
                    pf_tiles[t] = pf_pool.tile([128, TILE_F], f32, tag="pf", name="pf_t")
                    nc.sync.dma_start(pf_tiles[t][:], pf_ext[:, bass.ts(t, TILE_F)])
                pp_t, pf_t = pp_tiles.pop(t), pf_tiles.pop(t)
                ps = psum_pool.tile([128, PSUM_F], f32, tag="ps", name="ps")
                o_t = out_pool.tile([128, TILE_F], f32, tag="o", name="o_t")
                compute_span(pp_t, pf_t, ps, o_t, 0, PSUM_F, t, t * TILE_F)

    return nc


def _shard_transposed(x: np.ndarray, k: int) -> np.ndarray:
    """Shard rows [k*RPC, (k+1)*RPC) and lay out as [128, CHUNK] with
    partition 32*q + b = x[k*RPC + q*CHUNK + i, b]."""
    shard = x[k * RPC : (k + 1) * RPC, :]
    return np.ascontiguousarray(
        shard.reshape(NGRP, CHUNK, B).transpose(0, 2, 1).reshape(128, CHUNK)
    )


def kernel(pf: np.ndarray, pp: np.ndarray, cp: np.ndarray) -> np.ndarray:
    global LAST_EXEC_TIME_NS, LAST_RESULTS
    from concourse.bass_utils import run_bass_kernel_spmd

    pf = np.ascontiguousarray(np.asarray(pf, dtype=np.float32))
    pp = np.ascontiguousarray(np.asarray(pp, dtype=np.float32))
    cp = np.ascontiguousarray(np.asarray(cp, dtype=np.float32))

    cp_rep = np.ascontiguousarray(np.tile(cp, (NGRP, 1)))
    in_maps = [
        {
            "pft": _shard_transposed(pf, k),
            "ppt": _shard_transposed(pp, k),
            "cp": cp_rep,
        }
        for k in range(NCORES)
    ]

    nc = _build_nc()
    nc.finalize()
    trace = os.environ.get("KERNEL_TRACE", "0") == "1"
    if trace:
        _ensure_ntff_hook()
        # Skip the (slow, possibly unavailable) artifact upload.
        import concourse.bass_utils as _bu

        _bu.upload_artifacts = lambda tmpdir: "local://skipped"
    res = run_bass_kernel_spmd(
        nc, in_maps, core_ids=list(range(NCORES)), trace=trace
    )
    LAST_EXEC_TIME_NS = res.exec_time_ns
    LAST_RESULTS = res

    out = np.empty((I, J), dtype=np.float32)
    for k in range(NCORES):
        o = res.results[k]["out"]  # [128, CHUNK]
        out[k * RPC : (k + 1) * RPC, :] = (
            o.reshape(NGRP, B, CHUNK).transpose(0, 2, 1).reshape(RPC, J)
        )
    return out
